# revision 44
# baseline (speedup 1.0000x reference)
"""CoAttention kernel for 8 Trainium2 NeuronCores.

Problem: S, D: [8, 2048, 1024] f32.
  G_b = D_b @ S_b^T                         [2048, 2048]
  co_D = D + rowsoftmax(G) @ S
  co_S = S + rowsoftmax(G^T) @ D
Data-parallel over batch: one batch per core, same NEFF on all 8 cores.

Design (per core), v2:
  Phase A: load S; build S^T chunks (f32r, stage-1 rhs) via PE
           transposes; cast S16 (fp16 stage-2 rhs / phase-C residual).
           Block 0's stage-1 G chunks are woven in as S^T chunks land.
  Phase B (per 128-row l-block i): load D; cast D16; PE-transpose
           D -> D^T f32r; G chunks via f32r matmuls (fp32 logits);
           per-chunk PE-transpose of G -> running per-column max (DVE,
           negated, stored per (mblk, lblk) in Mst) and fp8e4m3
           exp(G^T - runmax) stored in SBUF (HT; no DRAM round trip);
           row softmax (ACT exp, fp16) -> E chunks; stage-2
           O_D = E^T-tiles @ S16 (fp16 matmuls) + D16 residual -> co_D.
           Stage-2 of block i runs during block i+1 (software pipeline)
           so PE never waits on the softmax.
  Phase C (per 128-row m-block j): weights = HT * combo where
           combo_i = exp(M_run@i - M_final) / colsum folds the exact
           column-softmax normalization (colsum via per-chunk sums of
           HT dotted with cfac); stage-2 O_S = E2^T-tiles @ D16 + S16
           residual -> co_S. Preludes run two blocks ahead.

Measured on HW: fp8 DoubleRow stage-2, f32r-identity transposes, and
any GPSIMD op with fp8/fp16 inputs are all much slower than this
fp16/fp32 configuration despite favorable cost-model numbers; the
DEFAULTS reflect the fast-on-hardware choices.
"""

import numpy as np

P = 128
T = 2048
DH = 1024
LT = T // P     # 16 token blocks per side
KD = DH // P    # 8 contraction blocks
NT = 512        # stage-1 moving free dim
NCH = T // NT   # 4 chunks per token row

DEFAULTS = dict(
    fp8_stage2=False,     # fp8 DoubleRow stage-2 is slower on real HW
    out_dma_eng="sync",   # engine issuing output DMAs
    rescale_eng="dve",    # phase-C weight rescale engine
    resid_eng="pool",     # residual-add engine
    stage_bufs=3,
    gsb_bufs=4,
    epool_bufs=5,
    etp_bufs=3,
    outp_bufs=2,
    dtp_bufs=2,
    small_bufs=4,
    tpsum_bufs=2,
    tpe_bufs=2,
    gpsum_bufs=2,
    opsum_bufs=2,
)

_CACHE = {}


def _build_nc(**overrides):
    import concourse.bass as bass
    import concourse.mybir as mybir
    import concourse.tile as tile
    from concourse import bacc
    from concourse.masks import make_identity

    p = dict(DEFAULTS)
    p.update(overrides)

    dt = mybir.dt
    f32, f32r = dt.float32, dt.float32r
    f16 = dt.float16
    f8 = dt.float8e4      # e4m3: stage-2 operands + exported exp weights
    AX = mybir.AxisListType.X
    EXP = mybir.ActivationFunctionType.Exp
    COPY = mybir.ActivationFunctionType.Copy
    MAX = mybir.AluOpType.max
    MIN = mybir.AluOpType.min
    SUB = mybir.AluOpType.subtract
    MULT = mybir.AluOpType.mult
    DR = mybir.MatmulPerfMode.DoubleRow

    use_fp8 = p["fp8_stage2"]
    e_dt = f8 if use_fp8 else f16

    nc = bacc.Bacc("TRN2", target_bir_lowering=False, debug=False)

    S_ap = nc.dram_tensor("S", [T, DH], f32, kind="ExternalInput").ap()
    D_ap = nc.dram_tensor("D", [T, DH], f32, kind="ExternalInput").ap()
    coD_ap = nc.dram_tensor("co_D", [T, DH], f32, kind="ExternalOutput").ap()
    coS_ap = nc.dram_tensor("co_S", [T, DH], f32, kind="ExternalOutput").ap()

    with tile.TileContext(nc) as tc:
        with (
            tc.tile_pool(name="consts", bufs=1) as consts,
            tc.tile_pool(name="big", bufs=1) as big,
            tc.tile_pool(name="stage", bufs=p["stage_bufs"]) as stage,
            tc.tile_pool(name="gsb", bufs=p["gsb_bufs"]) as gsb,
            tc.tile_pool(name="epool", bufs=p["epool_bufs"]) as epool,
            tc.tile_pool(name="etp", bufs=p["etp_bufs"]) as etp,
            tc.tile_pool(name="outp", bufs=p["outp_bufs"]) as outp,
            tc.tile_pool(name="dtp", bufs=p["dtp_bufs"]) as dtp,
            tc.tile_pool(name="small", bufs=p["small_bufs"]) as small,
            tc.tile_pool(name="tpsum", bufs=p["tpsum_bufs"], space="PSUM") as tpsum,
            tc.tile_pool(name="tpe", bufs=p["tpe_bufs"], space="PSUM") as tpe,
            tc.tile_pool(name="gpsum", bufs=p["gpsum_bufs"], space="PSUM") as gpsum,
            tc.tile_pool(name="opsum", bufs=p["opsum_bufs"], space="PSUM") as opsum,
        ):
            ident_f32 = consts.tile([P, P], f32)
            make_identity(nc, ident_f32[:])
            ident_e16 = consts.tile([P, P], f16)
            make_identity(nc, ident_e16[:])

            # [d%128, (dblk, m-chunk)] x 4 chunks; chunk mc holds S rows
            # [mc*NT, (mc+1)*NT) so stage 1 can start before all of S lands.
            S_T = [big.tile([P, KD, NT], f32r, name=f"st{c}") for c in range(NCH)]
            if use_fp8:
                S8hi = big.tile([P, LT, DH], f8)   # [m%128, (mblk, d)]
                S8lo = big.tile([P, LT, DH], f8)
                D8hi = big.tile([P, LT, DH], f8)   # [l%128, (lblk, d)]
                D8lo = big.tile([P, LT, DH], f8)
            else:
                S8hi = big.tile([P, LT, DH], f16)  # fp16 rhs, no split
                S8lo = None
                D8hi = big.tile([P, LT, DH], f16)
                D8lo = None
            # exp(G^T - runmax): [m%128, (mblk, l)]; rescaled in phase C
            HT = big.tile([P, LT, T], f8)
            Mst = big.tile([P, LT, LT], f32)   # [m%128, (mblk, lblk)] -runmax

            def transpose_128(src, psum_tile, k4):
                nc.tensor.transpose(psum_tile[:, k4, :], src, ident_f32[:])

            def build_T(st, dst_slices):
                """PE-transpose st [P, DH] f32 into dst [P,4,P] slices (x2)."""
                for g in range(2):
                    pt = tpsum.tile([P, 4, P], f32, tag="tp")
                    for k4 in range(4):
                        k = g * 4 + k4
                        transpose_128(st[:, k * P:(k + 1) * P], pt, k4)
                    nc.vector.tensor_copy(dst_slices(g), pt[:])

            def split8(src, hi_sl, lo_sl):
                nc.scalar.copy(hi_sl, src)
                if use_fp8:
                    nc.vector.tensor_tensor(lo_sl, src, hi_sl, op=SUB)

            def stage2(e_chunks, rhs_hi, rhs_lo, rscale, resid, out_ap,
                       et_copy_eng="act"):
                """O = rowscale * (E @ (rhs_hi+rhs_lo)) + resid -> out_ap.

                e_chunks: 4 fp16 [P, NT] tiles (softmax weights, one per
                512-col group). fp8 path: DoubleRow matmuls, each k-plane
                pair carrying adjacent m-blocks for both hi and lo rhs.
                rscale=None means weights are pre-normalized: emit is a
                single fused Pool add reading PSUM."""
                ets = []
                for g in range(2):
                    pt = tpe.tile([P, 8, P], f16, tag="tpe")
                    for k8 in range(8):
                        ch = e_chunks[2 * g + k8 // 4]
                        k4 = k8 % 4
                        nc.tensor.transpose(
                            pt[:, k8, :], ch[:, k4 * P:(k4 + 1) * P],
                            ident_e16[:],
                        )
                    et = etp.tile([P, 8, P], e_dt, tag="et")
                    if et_copy_eng == "dve":
                        nc.vector.tensor_copy(et[:], pt[:])
                    else:
                        nc.scalar.copy(et[:], pt[:])
                    ets.append(et)
                dma_eng = nc.sync if p["out_dma_eng"] == "sync" else nc.gpsimd
                for n in range(DH // NT):
                    ps = opsum.tile([P, NT], f32, tag="ps")
                    nsl = slice(n * NT, (n + 1) * NT)
                    if use_fp8:
                        for src_i, rhs in enumerate((rhs_hi, rhs_lo)):
                            for pk in range(LT // 2):
                                g, h = pk // 4, (pk % 4) * 2
                                nc.tensor.matmul(
                                    ps[:],
                                    ets[g][:, h:h + 2, :],
                                    rhs[:, 2 * pk:2 * pk + 2, nsl],
                                    start=(src_i == 0 and pk == 0),
                                    stop=(src_i == 1 and pk == LT // 2 - 1),
                                    perf_mode=DR,
                                )
                    else:
                        for kb in range(LT):
                            nc.tensor.matmul(
                                ps[:],
                                ets[kb // 8][:, kb % 8, :],
                                rhs_hi[:, kb, nsl],
                                start=(kb == 0),
                                stop=(kb == LT - 1),
                            )
                    o = outp.tile([P, NT], f32, tag="o")
                    r_aps = resid(nsl)
                    if rscale is not None:
                        nc.vector.tensor_scalar(
                            o[:], ps[:], rscale[:], None, op0=MULT
                        )
                        for r_ap in r_aps:
                            nc.vector.tensor_add(o[:], o[:], r_ap)
                    else:
                        # weights pre-normalized: single fused psum+resid add
                        nc.vector.tensor_tensor(
                            o[:], ps[:], r_aps[0], op=mybir.AluOpType.add
                        )
                        for r_ap in r_aps[1:]:
                            nc.vector.tensor_add(o[:], o[:], r_ap)
                    dma_eng.dma_start(out_ap[:, nsl], o[:])

            def export_gt(mc, g_c, i):
                ptg = tpsum.tile([P, 4, P], f32, tag="tp")
                for j4 in range(4):
                    transpose_128(g_c[:, j4 * P:(j4 + 1) * P], ptg, j4)
                # one strided reduce covers all 4 m-blocks: [P,4,l]->[P,4]
                jsl = slice(4 * mc, 4 * mc + 4)
                if i == 0:
                    nc.vector.reduce_max(
                        Mst[:, jsl, 0:1], ptg[:], axis=AX, negate=True
                    )
                else:
                    nbm = small.tile([P, 4, 1], f32, tag="nbm", name="nbm")
                    nc.vector.reduce_max(nbm[:], ptg[:], axis=AX, negate=True)
                    nc.vector.tensor_tensor(
                        Mst[:, jsl, i:i + 1], Mst[:, jsl, i - 1:i],
                        nbm[:], op=MIN,
                    )
                for j4 in range(4):
                    j = 4 * mc + j4
                    nc.scalar.activation(
                        HT[:, j, i * P:(i + 1) * P], ptg[:, j4, :], EXP,
                        bias=Mst[:, j, i:i + 1], scale=1.0,
                    )

            def g_chunk(dt_i, mc, i, rmp, g_chunks):
                """One [P,NT] chunk of G row-block i: matmuls, copy, rowmax."""
                gp = gpsum.tile([P, NT], f32, tag="g")
                for k in range(KD):
                    nc.tensor.matmul(
                        gp[:],
                        dt_i[:, k, :],
                        S_T[mc][:, k, :],
                        start=(k == 0),
                        stop=(k == KD - 1),
                    )
                g_c = gsb.tile([P, NT], f32, tag="g")
                nc.vector.tensor_copy(g_c[:], gp[:])
                nc.vector.tensor_reduce(
                    rmp[:, mc:mc + 1], g_c[:], axis=AX, op=MAX
                )
                g_chunks.append(g_c)

            def d_prelude(i, std):
                split8(std[:], D8hi[:, i, :], D8lo[:, i, :] if use_fp8 else None)
                dt_i = dtp.tile([P, KD, P], f32r)
                build_T(std, lambda g: dt_i[:, g * 4:(g + 1) * 4, :])
                return dt_i

            # ---- Phase A: S loads -> S^T chunks + S16/S8 splits, with
            # block 0 of stage 1 woven in as its S^T chunks become ready ----
            st_tiles = {}
            std_tiles = {}
            for i in range(2):
                st_tiles[i] = stage.tile([P, DH], f32, tag="ld", name="st")
                nc.sync.dma_start(st_tiles[i][:], S_ap[i * P:(i + 1) * P, :])
            rmp0 = small.tile([P, NCH], f32, tag="rmp", name="rmp")
            g_chunks0 = []
            dt_0 = None
            for i in range(LT):
                if i + 2 < LT:
                    st_tiles[i + 2] = stage.tile([P, DH], f32, tag="ld", name="st")
                    nc.sync.dma_start(
                        st_tiles[i + 2][:], S_ap[(i + 2) * P:(i + 3) * P, :]
                    )
                st = st_tiles.pop(i)
                split8(st[:], S8hi[:, i, :], S8lo[:, i, :] if use_fp8 else None)
                mc, r = i // 4, i % 4
                build_T(
                    st,
                    lambda g, mc=mc, r=r: S_T[mc][:, g * 4:(g + 1) * 4,
                                                  r * P:(r + 1) * P],
                )
                if i == 3:
                    std_tiles[0] = stage.tile([P, DH], f32, tag="ld", name="std")
                    nc.sync.dma_start(std_tiles[0][:], D_ap[0:P, :])
                if r == 3 and mc > 0:
                    g_chunk(dt_0, mc - 1, 0, rmp0, g_chunks0)
                    if mc > 1:
                        export_gt(mc - 2, g_chunks0[mc - 2], 0)
                if i == 4:
                    dt_0 = d_prelude(0, std_tiles.pop(0))
                if i == 14:
                    std_tiles[1] = stage.tile([P, DH], f32, tag="ld", name="std")
                    nc.sync.dma_start(std_tiles[1][:], D_ap[P:2 * P, :])

            # ---- Phase B ----
            pending = None  # deferred stage-2 of previous l-block

            for i in range(LT):
                if i + 2 < LT:
                    std_tiles[i + 2] = stage.tile([P, DH], f32, tag="ld", name="std")
                    nc.sync.dma_start(
                        std_tiles[i + 2][:], D_ap[(i + 2) * P:(i + 3) * P, :]
                    )
                if i == 0:
                    dt_i, rmp, g_chunks = dt_0, rmp0, g_chunks0
                    gt_defer = [2, 3]
                    g_chunk(dt_i, 3, 0, rmp, g_chunks)
                else:
                    std = std_tiles.pop(i)
                    dt_i = d_prelude(i, std)
                    rmp = small.tile([P, NCH], f32, tag="rmp", name="rmp")
                    g_chunks = []
                    gt_defer = []
                    for mc in range(NCH):
                        g_chunk(dt_i, mc, i, rmp, g_chunks)
                        gt_defer.append(mc)
                        if len(gt_defer) > 1:
                            mcd = gt_defer.pop(0)
                            export_gt(mcd, g_chunks[mcd], i)

                # previous block's stage-2 (pipelined behind this block's G)
                if pending is not None:
                    stage2(*pending)
                    pending = None

                for mcd in gt_defer:
                    export_gt(mcd, g_chunks[mcd], i)

                # row softmax
                nr = small.tile([P, 1], f32, tag="nr")
                nc.vector.reduce_max(nr[:], rmp[:], axis=AX, negate=True)
                rsp = small.tile([P, NCH], f32, tag="rsp", name="rsp")
                e_chunks = []
                for mc in range(NCH):
                    e_c = epool.tile([P, NT], f16, tag="e")
                    nc.scalar.activation(
                        e_c[:], g_chunks[mc][:], EXP, bias=nr[:], scale=1.0,
                        accum_out=rsp[:, mc:mc + 1],
                    )
                    e_chunks.append(e_c)
                rs = small.tile([P, 1], f32, tag="rs")
                nc.vector.reduce_sum(rs[:], rsp[:], axis=AX)
                rrs = small.tile([P, 1], f32, tag="rrs")
                nc.vector.reciprocal(rrs[:], rs[:])

                if use_fp8:
                    resid = (lambda nsl, i=i: [D8hi[:, i, nsl],
                                               D8lo[:, i, nsl]])
                else:
                    resid = lambda nsl, i=i: [D8hi[:, i, nsl]]
                pending = (e_chunks, S8hi, S8lo, rrs, resid,
                           coD_ap[i * P:(i + 1) * P, :], "act")

            stage2(*pending)
            pending = None

            # ---- Phase C: H^T -> col softmax -> O_S ----
            # fp16 mode: S16 rows double as the residual (no S reload)
            sst_tiles = {}
            if use_fp8:
                for j in range(2):
                    sst_tiles[j] = stage.tile([P, DH], f32, tag="ld", name="sst")
                    nc.sync.dma_start(
                        sst_tiles[j][:], S_ap[j * P:(j + 1) * P, :]
                    )

            def prelude_c(j):
                """combo_i = exp(M_run@i - M_final) / Z: per-chunk factors
                folding the exact colsum normalization."""
                biasC = small.tile([P, LT], f32, tag="bc", name="biasC")
                nc.vector.tensor_scalar(
                    biasC[:], Mst[:, j, :], Mst[:, j, LT - 1:LT], -1.0,
                    op0=SUB, op1=MULT,
                )
                cfac = small.tile([P, LT], f32, tag="cf", name="cfac")
                nc.scalar.activation(cfac[:], biasC[:], EXP)
                # Z = sum_i cfac_i * (per-chunk sums of stored weights)
                hs = small.tile([P, LT, 1], f32, tag="hs", name="hs")
                nc.vector.tensor_reduce(
                    hs[:], HT[:, j, :].rearrange("p (i l) -> p i l", l=P),
                    axis=AX, op=mybir.AluOpType.add,
                )
                zp = small.tile([P, LT], f32, tag="zp", name="zp")
                nc.vector.tensor_tensor(
                    zp[:], hs[:].rearrange("p i l -> p (i l)"), cfac[:], op=MULT
                )
                zs = small.tile([P, 1], f32, tag="rs")
                nc.vector.reduce_sum(zs[:], zp[:], axis=AX)
                rz = small.tile([P, 1], f32, tag="rrs")
                nc.vector.reciprocal(rz[:], zs[:])
                combo = small.tile([P, LT], f32, tag="cb", name="combo")
                nc.vector.tensor_scalar(
                    combo[:], cfac[:], rz[:], None, op0=MULT
                )
                return combo

            def rescales_c(j, combo):
                resc = nc.gpsimd if p["rescale_eng"] == "pool" else nc.vector
                e2 = []
                for g in range(NCH):
                    e_c = epool.tile([P, NT], f16, tag="e")
                    for i4 in range(4):
                        i = 4 * g + i4
                        resc.tensor_scalar(
                            e_c[:, i4 * P:(i4 + 1) * P],
                            HT[:, j, i * P:(i + 1) * P],
                            combo[:, i:i + 1], None, op0=MULT,
                        )
                    e2.append(e_c)
                return e2

            combos = {0: prelude_c(0), 1: prelude_c(1)}
            e2_cur = rescales_c(0, combos.pop(0))
            for j in range(LT):
                if use_fp8 and j + 2 < LT:
                    sst_tiles[j + 2] = stage.tile([P, DH], f32, tag="ld", name="sst")
                    nc.sync.dma_start(
                        sst_tiles[j + 2][:], S_ap[(j + 2) * P:(j + 3) * P, :]
                    )
                if j + 2 < LT:
                    combos[j + 2] = prelude_c(j + 2)
                e2_nxt = (rescales_c(j + 1, combos.pop(j + 1))
                          if j + 1 < LT else None)
                if use_fp8:
                    sst = sst_tiles.pop(j)
                    resid = lambda nsl, sst=sst: [sst[:, nsl]]
                else:
                    resid = lambda nsl, j=j: [S8hi[:, j, nsl]]
                stage2(e2_cur, D8hi, D8lo, None, resid,
                       coS_ap[j * P:(j + 1) * P, :], et_copy_eng="act")
                e2_cur = e2_nxt

    nc.compile()
    return nc


def _get_nc():
    if "nc" not in _CACHE:
        import json
        import os

        ov = json.loads(os.environ.get("KERNEL_OVERRIDES", "{}"))
        _CACHE["nc"] = _build_nc(**ov)
    return _CACHE["nc"]


def kernel(S, D):
    from concourse.bass_utils import run_bass_kernel_spmd

    S = np.ascontiguousarray(np.asarray(S, dtype=np.float32))
    D = np.ascontiguousarray(np.asarray(D, dtype=np.float32))
    B = S.shape[0]
    assert S.shape == (B, T, DH) and D.shape == (B, T, DH) and B == 8

    nc = _get_nc()
    in_maps = [{"S": S[b], "D": D[b]} for b in range(B)]
    res = run_bass_kernel_spmd(nc, in_maps, core_ids=list(range(B)))
    co_D = np.stack([res.results[b]["co_D"] for b in range(B)])
    co_S = np.stack([res.results[b]["co_S"] for b in range(B)])
    return (co_D, co_S)
